# revision 4
# baseline (speedup 1.0000x reference)
"""Expert-parallel MoE kernel for one TRN2 chip (8 NeuronCores).

nn_DynamicRouterMoE: B=4, T=2048, C=1024, E=16, H=4096, top-2 routing.

v3: router/top-2/softmax/dispatch on the HOST (fp64 -> exact ordering vs
the fp32 reference; min top-2 logit gap ~1e-5 >> fp64 error). The device
runs a pure FFN per core over SLOTS of host-compacted fp16 token panels.

Load balance under the SPMD constraint (all cores run one program, so
panel capacities are static): each expert's token list is split into two
groups; the 32 groups are binned into 4 "bands" of 8 (one group per core
per band). Band capacities come from a small search minimizing the total
(~2072 slots/core vs 2048 ideal vs 2176 for whole-expert pairing).

Per core, per slot s (ascending capacity so the first xg DMA is small):
  xg[s]: [128(c), CC, cap_s] fp16 panel (host-gathered, transposed)
  for hc in 8 chunks of HC=512 over H (w1/w2 streamed, 2 MB/chunk):
    hT = relu(xg @ w1_chunk + b1)   (PE fp16 -> PSUM, Scalar relu)
    yT += hT @ w2_chunk             (PE fp16, Vector accumulate fp32)
  yT -> HBM raw (channel-major); host adds b2, gates, scatter-adds.

PE roofline: 2072 slots x 512 MAC-cycles @2.45 GHz ~= 433 us.
"""

from contextlib import ExitStack
from itertools import combinations_with_replacement

import numpy as np

import concourse.bacc as bacc
import concourse.mybir as mybir
from concourse import bass_utils
from concourse.tile import TileContext

dt = mybir.dt
AF = mybir.ActivationFunctionType

# problem shape (hardcoded per contest contract)
B, T, C, E, H = 4, 2048, 1024, 16, 4096
N = B * T                  # 8192 tokens
NCORES = 8
NBANDS = 4                 # slots (token panels) per core
HC = 512                   # H chunk streamed from HBM
CC = C // 128              # 8 contraction chunks
NHC = H // HC              # 8 H chunks
HT = HC // 128             # 4
MOVW = 512                 # moving-operand tile width (tokens per matmul)

_NC_CACHE = {}
_LAST_META = {}


def _build(caps):
    """caps: ascending static token capacities of the NBANDS slots."""
    nc = bacc.Bacc("TRN2", target_bir_lowering=False, debug=False,
                   num_devices=NCORES)
    xgd = [nc.dram_tensor(f"xg{s}", [CC, 128, cap], dt.float16,
                          kind="ExternalInput") for s, cap in enumerate(caps)]
    w1 = nc.dram_tensor("w1", [NBANDS, C, H], dt.float16, kind="ExternalInput")
    w2 = nc.dram_tensor("w2", [NBANDS, H, C], dt.float16, kind="ExternalInput")
    b1 = nc.dram_tensor("b1", [NBANDS, H], dt.float32, kind="ExternalInput")
    ytd = [nc.dram_tensor(f"yt{s}", [CC, 128, cap], dt.float32,
                          kind="ExternalOutput") for s, cap in enumerate(caps)]

    with TileContext(nc) as tc, ExitStack() as ctx:
        const_pool = ctx.enter_context(tc.tile_pool(name="const", bufs=1))
        xg_pool = ctx.enter_context(tc.tile_pool(name="xg", bufs=1))
        w_pool = ctx.enter_context(tc.tile_pool(name="w", bufs=2))
        h_pool = ctx.enter_context(tc.tile_pool(name="h", bufs=2))
        yacc_pool = ctx.enter_context(tc.tile_pool(name="yacc", bufs=1))
        psh_pool = ctx.enter_context(tc.tile_pool(name="psh", bufs=3, space="PSUM"))
        psy_pool = ctx.enter_context(tc.tile_pool(name="psy", bufs=3, space="PSUM"))

        for s, cap in enumerate(caps):
            tiles = [(o, min(MOVW, cap - o)) for o in range(0, cap, MOVW)]

            xg = xg_pool.tile([128, CC, cap], dt.float16, tag=f"xg{s}",
                              name=f"xg{s}")
            nc.sync.dma_start(xg[:, :, :], xgd[s].rearrange("cc p t -> p cc t"))
            b1s = const_pool.tile([128, H // 128], dt.float32, tag=f"b1{s}",
                                  name=f"b1{s}")
            nc.sync.dma_start(b1s[:, :],
                              b1[s].rearrange("(ht p) -> p ht", p=128))

            yT = yacc_pool.tile([128, CC, cap], dt.float32, tag=f"yT{s}",
                                name=f"yT{s}")

            for hc in range(NHC):
                w1c = w_pool.tile([128, CC * HC], dt.float16, tag="w1c")
                nc.sync.dma_start(
                    w1c.rearrange("p (cc h) -> p cc h", h=HC),
                    w1[s, :, hc * HC:(hc + 1) * HC]
                    .rearrange("(cc p) h -> p cc h", p=128))
                w2c = w_pool.tile([128, HT * C], dt.float16, tag="w2c")
                nc.sync.dma_start(
                    w2c.rearrange("p (ht ck) -> p ht ck", ck=C),
                    w2[s, hc * HC:(hc + 1) * HC, :]
                    .rearrange("(ht p) ck -> p ht ck", p=128))

                hT = h_pool.tile([128, HT, cap], dt.float16, tag="hT")
                # h = relu(x @ w1c + b1): tile-outer so the last relu is off
                # the PE critical path when the y-phase starts
                for off, wd in tiles:
                    for ht in range(HT):
                        ps_h = psh_pool.tile([128, MOVW], dt.float32, tag="psh")
                        for cc in range(CC):
                            nc.tensor.matmul(
                                ps_h[:, 0:wd],
                                w1c[:, cc * HC + ht * 128:cc * HC + (ht + 1) * 128],
                                xg[:, cc, off:off + wd],
                                start=(cc == 0), stop=(cc == CC - 1))
                        nc.scalar.activation(
                            hT[:, ht, off:off + wd], ps_h[:, 0:wd],
                            AF.Relu,
                            bias=b1s[:, hc * HT + ht:hc * HT + ht + 1])
                # y += h @ w2c: ct-outer on the last chunk so each finished
                # output strip DMAs out while the rest still computes
                for ct in range(CC):
                    for off, wd in tiles:
                        ps_y = psy_pool.tile([128, MOVW], dt.float32, tag="psy")
                        for ht in range(HT):
                            nc.tensor.matmul(
                                ps_y[:, 0:wd],
                                w2c[:, ht * C + ct * 128:ht * C + (ct + 1) * 128],
                                hT[:, ht, off:off + wd],
                                start=(ht == 0), stop=(ht == HT - 1))
                        if hc == 0:
                            nc.vector.tensor_copy(yT[:, ct, off:off + wd],
                                                  ps_y[:, 0:wd])
                        else:
                            nc.vector.tensor_add(
                                yT[:, ct, off:off + wd],
                                yT[:, ct, off:off + wd], ps_y[:, 0:wd])
                    if hc == NHC - 1:
                        nc.sync.dma_start(ytd[s][ct, :, :], yT[:, ct, :])

    nc.compile()
    return nc


def _route_host(x, w_router):
    """Exact top-2 routing on host (fp64; reference fp32 gap ~1e-5)."""
    xf = np.ascontiguousarray(np.asarray(x, dtype=np.float64).reshape(N, C))
    wr = np.asarray(w_router, dtype=np.float64)
    logits = xf @ wr                                     # [N, E]
    sel = np.argpartition(logits, E - 2, axis=1)[:, -2:]  # top2, unordered
    lv = np.take_along_axis(logits, sel, axis=1)
    swap = lv[:, 0] < lv[:, 1]
    sel[swap] = sel[swap][:, ::-1]
    lv[swap] = lv[swap][:, ::-1]
    # softmax over the two logits
    d = np.exp(lv[:, 1] - lv[:, 0])
    p0 = 1.0 / (1.0 + d)
    probs = np.stack([p0, 1.0 - p0], axis=1).astype(np.float32)  # [N, 2]
    return sel.astype(np.int64), probs


def _band_plan(counts):
    """Split each expert's token count into 2 groups binned into NBANDS
    bands of NCORES groups; minimize total band capacities (greedy-checked
    capacity search). Returns (caps ascending, plan) where
    plan[band][core] = (expert, start, size)."""
    order = np.argsort(-counts, kind="stable")

    def assign(v):
        loads = [0] * NBANDS
        out = []
        for e in order:
            c = int(counts[e])
            best = None
            for i, j in combinations_with_replacement(range(NBANDS), 2):
                if i == j and loads[i] + 2 > NCORES:
                    continue
                if i != j and (loads[i] + 1 > NCORES or loads[j] + 1 > NCORES):
                    continue
                if v[i] + v[j] < c:
                    continue
                w = v[i] + v[j] - c
                if best is None or w < best[0]:
                    best = (w, i, j)
            if best is None:
                return None
            _, i, j = best
            loads[i] += 1
            loads[j] += 1
            out.append((e, i, j))
        return out

    # capacities stay multiples of 16: a 16-misaligned moving operand costs
    # ~25% extra PE time per matmul (SBUF line granularity)
    lo = int(np.ceil(counts.sum() / (NBANDS * NCORES) / 16) * 16)
    hi = int(np.ceil(counts.max() / 16) * 16) + 64
    grid = sorted(range(lo, hi + 1, 16), reverse=True)
    best = None
    for v in combinations_with_replacement(grid, NBANDS):
        v = tuple(sorted(v, reverse=True))
        if best and sum(v) >= best[0]:
            continue
        if assign(v) is not None:
            best = (sum(v), v)
    v = tuple(sorted(best[1]))            # ascending caps
    asg = assign(tuple(sorted(v, reverse=True)))
    # map band index of the search (desc order) to ascending slot index
    remap = {i: NBANDS - 1 - i for i in range(NBANDS)}
    plan = [[None] * NCORES for _ in range(NBANDS)]
    fill = [0] * NBANDS
    for e, i, j in asg:
        bi, bj = remap[i], remap[j]
        c = int(counts[e])
        gj = min(v[bj], c)
        gi = c - gj
        for b, start, size in ((bj, 0, gj), (bi, gj, gi)):
            plan[b][fill[b]] = (e, start, size)
            fill[b] += 1
    for b in range(NBANDS):
        while fill[b] < NCORES:
            plan[b][fill[b]] = (0, 0, 0)
            fill[b] += 1
    return v, plan


def prepare_in_maps(x, w_router, w1, b1, w2, b2):
    x = np.asarray(x, dtype=np.float32)
    w1 = np.asarray(w1, dtype=np.float32)
    b1 = np.asarray(b1, dtype=np.float32)
    w2 = np.asarray(w2, dtype=np.float32)

    sel, probs = _route_host(x, w_router)

    # per-expert compact token lists + gates
    flat_e = sel.ravel()                       # [2N] expert ids
    flat_t = np.repeat(np.arange(N), 2)        # token ids
    flat_g = probs.ravel()
    order = np.argsort(flat_e, kind="stable")
    counts = np.bincount(flat_e, minlength=E)
    starts = np.concatenate([[0], np.cumsum(counts)])
    tok_by_e = [flat_t[order[starts[e]:starts[e + 1]]] for e in range(E)]
    gate_by_e = [flat_g[order[starts[e]:starts[e + 1]]] for e in range(E)]

    caps, plan = _band_plan(counts)

    xf16T = np.ascontiguousarray(
        x.reshape(N, C).T.astype(np.float16))      # [C, N]
    w1_16 = w1.astype(np.float16)
    w2_16 = w2.astype(np.float16)

    in_maps = []
    for c in range(NCORES):
        ex = [plan[s][c][0] for s in range(NBANDS)]
        im = {
            "w1": np.ascontiguousarray(w1_16[ex]),
            "w2": np.ascontiguousarray(w2_16[ex]),
            "b1": np.ascontiguousarray(b1[ex]),
        }
        for s in range(NBANDS):
            e, g0, gn = plan[s][c]
            idx = tok_by_e[e][g0:g0 + gn]
            full = np.concatenate(
                [idx, np.zeros(caps[s] - gn, dtype=np.int64)])
            im[f"xg{s}"] = np.ascontiguousarray(
                xf16T[:, full].reshape(CC, 128, caps[s]))
        in_maps.append(im)

    _LAST_META.update(dict(caps=caps, plan=plan, tok_by_e=tok_by_e,
                           gate_by_e=gate_by_e, counts=counts))
    if caps not in _NC_CACHE:
        _NC_CACHE[caps] = _build(caps)
    _NC_CACHE["nc"] = _NC_CACHE[caps]
    return in_maps


def combine(results, b2):
    m = _LAST_META
    b2 = np.asarray(b2, dtype=np.float32)
    out = np.zeros((N, C), dtype=np.float32)
    for c in range(NCORES):
        r = results[c]
        for s in range(NBANDS):
            e, g0, gn = m["plan"][s][c]
            if gn == 0:
                continue
            idx = m["tok_by_e"][e][g0:g0 + gn]
            g = m["gate_by_e"][e][g0:g0 + gn]
            # y[tok_slot, ct*128+p] = yt[ct, p, slot]
            y = r[f"yt{s}"].transpose(2, 0, 1).reshape(m["caps"][s], C)[:gn]
            # tokens unique within one expert group -> fancy-index add
            out[idx] += (y + b2[e][None, :]) * g[:, None]
    return out.reshape(B, T, C)


def kernel(x, w_router, w1, b1, w2, b2):
    in_maps = prepare_in_maps(x, w_router, w1, b1, w2, b2)
    nc = _NC_CACHE["nc"]
    res = bass_utils.run_bass_kernel_spmd(nc, in_maps, core_ids=list(range(NCORES)))
    kernel.last_results = res
    return combine(res.results, np.asarray(b2, dtype=np.float32))
